# revision 41
# baseline (speedup 1.0000x reference)
"""Trainium2 Bass kernel for block-local MultiHeadAttention + output projection.

Reference computation (per batch b):
  Q = x @ Wq.T ; K = x @ Wk.T ; V = x @ Wv.T          x: [B, S=8192, 64]
  reshape to [B, G=512, H=16, 64] (token t = g*16 + h)
  E[g,h,k] = Q[g,h,:] . K[g,k,:]                      (16x16 block-diag attention)
  A = softmax(E / 32, axis=k)
  O[g,h,:] = sum_k A[g,h,k] V[g,k,:]
  out2[b, r, gm*64+d] = O[g=(gq,gm), h, d]  with r = h*32+gq, g = gq*16+gm
  y = out2 @ Wo.T + bo                                y: [B, 512, 1024]

v2 strategy (data-parallel over batch, 4 batches/core on 8 cores):
  - HOST pre-stages x into the two SBUF layouts the PE needs (bf16):
      XPP [p=k*8+j, (b,gm,q,d)]   token-major slabs (U-matmul stationary)
      XT2 [p=(q%2)*64+d, (b,gm,q//2,k,j)] feature-major slab pairs (E operands)
    and pre-folds weights: MT2 = blockdiag(Wk^T Wq x2), WoV = Wo_blk @ Wv,
    mask = kron(ones16, eye8). No on-device transposes or casts at all.
  - ZT = MT2 @ XT2 (block-diag, full 128-contract matmuls)
  - per (b, gm): E^T-psum (4 matmuls, q-parity row-packed), exp (scalar),
    mask-mul (gpsimd), U^T matmuls (col-half per gm parity), den matmul
    (ones stationary, 512-col stream per gm)
  - rden = reciprocal_approx_fast(den) (DVE), out2^T = U^T * rden fused into
    the OUT2T eviction
  - fc: y-tile = bias-matmul + sum_c2 (OUT2T-chunk stationary) @ WOVT
"""

import numpy as np
from contextlib import ExitStack

import concourse.bass as bass
import concourse.bacc as bacc
import concourse.mybir as mybir
import concourse.tile as tile

N_CORES = 8
B_GLOB = 32
B_LOC = B_GLOB // N_CORES   # 4 batches per core
SB = 8192                   # tokens per batch
D = 64                      # head dim
NG = 16                     # gm values (heads)
NQ = 4                      # q per batch-row-group
NJ = 8                      # groups per slab
NH = 16                     # tokens per group
E = 1024
RB = 512                    # out2 rows per batch
NSLAB = NG * NQ             # 64 slabs per batch
XCOL = NSLAB * D            # 4096 XPP/XT2 cols per batch

BF = mybir.dt.bfloat16
F32 = mybir.dt.float32
AF = mybir.ActivationFunctionType


def emit_body(ctx, tc, ins, outs, dbg, stage=99):
    nc = tc.nc
    xpp, xt2, wovt, mt2, maskc, bobf = ins
    y = outs["y"]

    # ---------------- persistent tensors ----------------
    pp = ctx.enter_context(tc.tile_pool(name="persist", bufs=1))
    XPP = pp.tile([128, B_LOC * XCOL], BF, tag="XPP")
    XT2 = pp.tile([128, B_LOC * XCOL], BF, tag="XT2")
    ZT = pp.tile([128, B_LOC * XCOL], BF, tag="ZT")
    WOVT = pp.tile([128, 8 * E], BF, tag="WOVT")
    OUT2T = pp.tile([128, B_LOC * 8 * RB], BF, tag="OUT2T")
    MASKT = pp.tile([128, 512], BF, tag="MASKT")
    MT2T = pp.tile([128, 128], BF, tag="MT2T")
    BOBF = pp.tile([1, E], BF, tag="BOBF")
    ONES64 = pp.tile([128, D], BF, tag="ONES64")
    ONESROW = pp.tile([1, 512], BF, tag="ONESROW")

    nc.vector.memset(ONES64[:], 1.0)
    nc.vector.memset(ONESROW[:], 1.0)
    # x loads on the SP HWDGE ring (batch order: Z_0 can start ~3us in);
    # weights go on the ACT ring so they don't delay batch 0. Batch 0 is
    # split fine so the first Z chunks / U matmuls start as early as possible.
    for b in range(B_LOC):
        nc.sync.dma_start(XT2[:, b * XCOL:(b + 1) * XCOL], xt2[b])
        nc.sync.dma_start(XPP[:, b * XCOL:(b + 1) * XCOL], xpp[b])
    nc.scalar.dma_start(MT2T[:], mt2)
    nc.scalar.dma_start(MASKT[:], maskc)
    nc.scalar.dma_start(WOVT[:], wovt)
    nc.scalar.dma_start(BOBF[:], bobf.rearrange("(p n) -> p n", p=1))

    if stage < 2:
        return

    # ---------------- pools ----------------
    # PSUM budget (8 banks): eps-pair 2 (bufs=1 x [128,1024]), ops 2 (zps
    # shares), dps 2, fc 2.
    eps_pool = ctx.enter_context(tc.tile_pool(name="eps", bufs=1, space="PSUM"))
    ops_pool = ctx.enter_context(tc.tile_pool(name="ops", bufs=3, space="PSUM"))
    dps_pool = ctx.enter_context(tc.tile_pool(name="dps", bufs=1, space="PSUM"))
    fc_pool = ctx.enter_context(tc.tile_pool(name="fcps", bufs=2, space="PSUM"))
    aex_pool = ctx.enter_context(tc.tile_pool(name="aex", bufs=6))
    am_pool = ctx.enter_context(tc.tile_pool(name="am", bufs=6))
    rden_pool = ctx.enter_context(tc.tile_pool(name="rden", bufs=3))
    fout_pool = ctx.enter_context(tc.tile_pool(name="fout", bufs=2))
    fpart_pool = ctx.enter_context(tc.tile_pool(name="fpart", bufs=1))

    # PE warmup: dummy matmuls fill the initial input-DMA wait and keep the
    # HAM clock gate at 8/8 before the real work lands
    for w in range(16):
        wps = fc_pool.tile([128, 512], F32, tag="fcps")
        nc.tensor.matmul(wps[:], ONESROW[:, 0:128], ONESROW[:],
                         start=True, stop=True)

    fout_state = {}
    fc_parts = [None] * 8

    def fc_half(b, idx, c2s=range(8), partial=None, add=None):
        # one fc half-tile: idx = rt*2 + he; y-DMA fires after he==1.
        # c2s: which OUT2T chunks to accumulate. partial: SBUF tile to stash
        # a partial sum in. add: SBUF partial to add during the final evict.
        rt, he = idx // 2, idx % 2
        fps = fc_pool.tile([128, 512], F32, tag="fcps")
        c2s = list(c2s)
        first = add is not None   # bias already in the partial being added
        if not first:
            nc.tensor.matmul(fps[:], ONESROW[:, 0:128],
                             BOBF[:, he * 512:(he + 1) * 512],
                             start=True, stop=False)
        for c2 in c2s:
            sec = (b * 8 + c2) * 512
            nc.tensor.matmul(
                fps[:],
                OUT2T[:, sec + rt * 128: sec + (rt + 1) * 128],
                WOVT[:, c2 * E + he * 512: c2 * E + he * 512 + 512],
                start=first, stop=(c2 == c2s[-1]),
            )
            first = False
        if partial is not None:
            nc.scalar.copy(partial[:], fps[:])
            return
        if he == 0:
            fo = fout_pool.tile([128, E], F32, tag="fout")
            fout_state[b] = fo
        fo = fout_state[b]
        if add is not None:
            nc.vector.tensor_add(fo[:, he * 512:(he + 1) * 512], fps[:], add[:])
        else:
            nc.scalar.copy(fo[:, he * 512:(he + 1) * 512], fps[:])
        if he == 1:
            row = b * RB + rt * 128
            nc.sync.dma_start(y[row:row + 128, :], fo[:])

    def z_chunk(b, r):
        zps = ops_pool.tile([128, 512], F32, tag="ops")
        nc.tensor.matmul(zps[:], MT2T[:],
                         XT2[:, b * XCOL + r * 512: b * XCOL + (r + 1) * 512],
                         start=True, stop=True)
        nc.any.tensor_copy(ZT[:, b * XCOL + r * 512: b * XCOL + (r + 1) * 512],
                           zps[:])

    for b in range(B_LOC):
        for r in range(XCOL // 512):
            z_chunk(b, r)
        if stage < 3:
            continue

        # ---------------- attention main loop ----------------
        # Column order within a gm tile is (qpar, qhi, h, j): q = qhi*2+qpar
        # lives at aoff(q) = (q%2)*256 + (q//2)*128 (E psum is parity-banked).
        for c in range(NG // 2):        # gm pairs
            dps = dps_pool.tile([128, 512], F32, tag="dps")
            ops = ops_pool.tile([128, 512], F32, tag="ops")
            # eps pair tile: 2 banks; bank=q-parity, cols gmh*256+(q//2)*128
            eps = eps_pool.tile([128, 1024], F32, tag="eps")
            ams = [None, None]
            for gmh in range(2):
                gm = c * 2 + gmh
                for q in range(NQ):
                    half = (q % 2) * 64
                    blk = b * XCOL + (gm * 2 + q // 2) * 128
                    col = (q % 2) * 512 + gmh * 256 + (q // 2) * 128
                    nc.tensor.matmul(
                        eps[:, col:col + 128],
                        ZT[half:half + 64, blk:blk + 128],
                        XT2[half:half + 64, blk:blk + 128],
                        start=True, stop=True,
                        tile_position=(half, 0),
                    )
                if stage < 4:
                    continue
                aex = aex_pool.tile([128, 512], BF, tag="aex")
                eview = eps[:].rearrange("p (par g cc) -> g p par cc",
                                         par=2, g=2)[gmh]
                nc.scalar.activation(aex[:], eview, AF.Exp, scale=1.0 / 32.0)
                am = am_pool.tile([128, 512], BF, tag="am")
                # gm0 mask on gpsimd (slow, overlaps gm1's exp); gm1 on DVE
                if gmh == 0:
                    nc.gpsimd.tensor_mul(am[:], aex[:], MASKT[:])
                else:
                    nc.vector.tensor_mul(am[:], aex[:], MASKT[:])
                ams[gmh] = am
            # fc filler sits exactly in the exp->mask chain wait on the PE
            # FIFO: batch b-1's fc half between this pair's E and U matmuls
            if stage >= 7 and b > 0:
                fc_half(b - 1, c)
            if stage >= 7 and b == B_LOC - 1 and c >= 4:
                for k in range(2):
                    idx = (c - 4) * 2 + k
                    part = fpart_pool.tile([128, 512], F32, tag="fpart%d" % idx)
                    fc_parts[idx] = part
                    fc_half(b, idx, c2s=range(4), partial=part)
            if stage < 5:
                continue
            # U^T matmuls, gm1 first (its DVE mask finishes first)
            for gmh in (1, 0):
                gm = c * 2 + gmh
                pb = gmh * 64
                am = ams[gmh]
                for q in range(NQ):
                    slab = (b * NG + gm) * NQ + q
                    aoff = (q % 2) * 256 + (q // 2) * 128
                    nc.tensor.matmul(
                        ops[pb:pb + 64, aoff:aoff + 128],
                        XPP[:, slab * D:(slab + 1) * D],
                        am[:, aoff:aoff + 128],
                        start=True, stop=True,
                        tile_position=(0, pb),
                    )
                # den matmul: single 512-col stream per gm
                nc.tensor.matmul(dps[pb:pb + 64, :], ONES64[:], am[:],
                                 start=True, stop=True, tile_position=(0, pb))
            if stage < 6:
                continue
            rden = rden_pool.tile([128, 512], F32, tag="rden")
            nc.vector.reciprocal_approx_fast(rden[:], dps[:])
            sec = (b * 8 + c) * 512
            # out2 row r = h*32 + q*8 + j, src col = qpar*256+qhi*128+h*8+j
            # (split by qpar: codegen handles at most 3 free dims per AP)
            for qpar in range(2):
                out_ap = OUT2T[:, sec:sec + 512].rearrange(
                    "p (h qhi qpar j) -> qpar p qhi h j",
                    h=NH, qhi=2, qpar=2, j=NJ)[qpar]
                nc.vector.tensor_mul(out_ap, ops[:, qpar * 256:qpar * 256 + 256],
                                     rden[:, qpar * 256:qpar * 256 + 256])
        if stage < 7:
            continue

    # epilogue: last batch's fc, second half (c2 4..7) + add stashed partials
    if stage >= 7:
        for idx in range(8):
            fc_half(B_LOC - 1, idx, c2s=range(4, 8), add=fc_parts[idx])

    # ---------------- debug dumps ----------------
    for name, T in (("xt2", XT2), ("zt", ZT), ("out2t", OUT2T)):
        if name in dbg:
            nc.sync.dma_start(dbg[name], T[:])


def build(reps=1, debug=(), stage=99):
    nc = bacc.Bacc("TRN2", target_bir_lowering=False, debug=False,
                   num_devices=N_CORES)
    xpp = nc.dram_tensor("xpp", [B_LOC, 128, XCOL], BF, kind="ExternalInput").ap()
    xt2 = nc.dram_tensor("xt2", [B_LOC, 128, XCOL], BF, kind="ExternalInput").ap()
    wovt = nc.dram_tensor("wovt", [128, 8 * E], BF, kind="ExternalInput").ap()
    mt2 = nc.dram_tensor("mt2", [128, 128], BF, kind="ExternalInput").ap()
    maskc = nc.dram_tensor("maskc", [128, 512], BF, kind="ExternalInput").ap()
    bobf = nc.dram_tensor("bobf", [E], BF, kind="ExternalInput").ap()
    y = nc.dram_tensor("y", [B_LOC * RB, E], F32, kind="ExternalOutput").ap()
    dbg = {}
    for name, shape, dt in [
        ("xt2", [128, B_LOC * XCOL], BF),
        ("zt", [128, B_LOC * XCOL], BF),
        ("out2t", [128, B_LOC * 8 * RB], BF),
    ]:
        if name in debug:
            dbg[name] = nc.dram_tensor(name, shape, dt, kind="ExternalOutput").ap()

    ins = (xpp, xt2, wovt, mt2, maskc, bobf)
    outs = {"y": y}
    with tile.TileContext(nc) as tc:
        with ExitStack() as ctx:
            if reps > 1:
                with tc.For_i(0, reps, 1):
                    emit_body(ctx, tc, ins, outs, dbg, stage=stage)
            else:
                emit_body(ctx, tc, ins, outs, dbg, stage=stage)
    nc.compile()
    return nc


def _bf(a):
    import ml_dtypes
    return np.asarray(a, dtype=np.float32).astype(ml_dtypes.bfloat16)


def prepare_in_maps(x, Wq, Wk, Wv, Wo, bo):
    """Host-side staging: layout x shards + fold weights. Returns in_maps."""
    x = np.asarray(x, np.float32)
    Wq = np.asarray(Wq, np.float32)
    Wk = np.asarray(Wk, np.float32)
    Wv = np.asarray(Wv, np.float32)
    Wo = np.asarray(Wo, np.float32)
    bo = np.asarray(bo, np.float32)

    # weights (shared across cores)
    MT = Wk.T @ Wq                      # Z = X @ MT so that E^T = Z X^T
    mt2 = np.zeros((128, 128), np.float32)
    mt2[:64, :64] = MT
    mt2[64:, 64:] = MT
    mt2 = _bf(mt2)
    maskc = _bf(np.tile(np.kron(np.ones((16, 16), np.float32),
                                np.eye(8, dtype=np.float32)), (1, 4)))
    # wov[gm][e, di] = sum_dv Wo[e, gm*64+dv] * Wv[dv, di]
    wov = np.einsum('gev,vd->ged', Wo.reshape(E, NG, D).transpose(1, 0, 2), Wv)
    # wovt[(gm%2)*64 + di, (gm//2)*1024 + e]
    wovt = _bf(np.ascontiguousarray(
        wov.reshape(8, 2, E, D).transpose(1, 3, 0, 2).reshape(128, 8 * E)))
    bobf = _bf(bo)

    in_maps = []
    for core in range(N_CORES):
        xs = x[core * B_LOC:(core + 1) * B_LOC]
        xr = xs.reshape(B_LOC, NQ, NJ, NG, NH, D)       # b q j gm k d
        xpp = _bf(np.ascontiguousarray(
            xr.transpose(0, 4, 2, 3, 1, 5).reshape(B_LOC, 128, XCOL)))
        xr2 = xs.reshape(B_LOC, 2, 2, NJ, NG, NH, D)    # b qhi qpar j gm k d
        xt2 = _bf(np.ascontiguousarray(
            xr2.transpose(0, 2, 6, 4, 1, 5, 3).reshape(B_LOC, 128, XCOL)))
        in_maps.append({
            "xpp": xpp, "xt2": xt2, "wovt": wovt, "mt2": mt2,
            "maskc": maskc, "bobf": bobf,
        })
    return in_maps


def kernel(x, Wq, Wk, Wv, Wo, bo):
    """Full-input entry point: shards batch over 8 cores, returns full output."""
    from concourse.bass_utils import run_bass_kernel_spmd

    nc = build()
    in_maps = prepare_in_maps(x, Wq, Wk, Wv, Wo, bo)
    res = run_bass_kernel_spmd(nc, in_maps, list(range(N_CORES)))
    out = np.concatenate([res.results[c]["y"] for c in range(N_CORES)], axis=0)
    return out.reshape(B_GLOB, RB, E)


# revision 42
# speedup vs baseline: 1.0064x; 1.0064x over previous
"""Trainium2 Bass kernel for block-local MultiHeadAttention + output projection.

Reference computation (per batch b):
  Q = x @ Wq.T ; K = x @ Wk.T ; V = x @ Wv.T          x: [B, S=8192, 64]
  reshape to [B, G=512, H=16, 64] (token t = g*16 + h)
  E[g,h,k] = Q[g,h,:] . K[g,k,:]                      (16x16 block-diag attention)
  A = softmax(E / 32, axis=k)
  O[g,h,:] = sum_k A[g,h,k] V[g,k,:]
  out2[b, r, gm*64+d] = O[g=(gq,gm), h, d]  with r = h*32+gq, g = gq*16+gm
  y = out2 @ Wo.T + bo                                y: [B, 512, 1024]

v2 strategy (data-parallel over batch, 4 batches/core on 8 cores):
  - HOST pre-stages x into the two SBUF layouts the PE needs (bf16):
      XPP [p=k*8+j, (b,gm,q,d)]   token-major slabs (U-matmul stationary)
      XT2 [p=(q%2)*64+d, (b,gm,q//2,k,j)] feature-major slab pairs (E operands)
    and pre-folds weights: MT2 = blockdiag(Wk^T Wq x2), WoV = Wo_blk @ Wv,
    mask = kron(ones16, eye8). No on-device transposes or casts at all.
  - ZT = MT2 @ XT2 (block-diag, full 128-contract matmuls)
  - per (b, gm): E^T-psum (4 matmuls, q-parity row-packed), exp (scalar),
    mask-mul (gpsimd), U^T matmuls (col-half per gm parity), den matmul
    (ones stationary, 512-col stream per gm)
  - rden = reciprocal_approx_fast(den) (DVE), out2^T = U^T * rden fused into
    the OUT2T eviction
  - fc: y-tile = bias-matmul + sum_c2 (OUT2T-chunk stationary) @ WOVT
"""

import numpy as np
from contextlib import ExitStack

import concourse.bass as bass
import concourse.bacc as bacc
import concourse.mybir as mybir
import concourse.tile as tile

N_CORES = 8
B_GLOB = 32
B_LOC = B_GLOB // N_CORES   # 4 batches per core
SB = 8192                   # tokens per batch
D = 64                      # head dim
NG = 16                     # gm values (heads)
NQ = 4                      # q per batch-row-group
NJ = 8                      # groups per slab
NH = 16                     # tokens per group
E = 1024
RB = 512                    # out2 rows per batch
NSLAB = NG * NQ             # 64 slabs per batch
XCOL = NSLAB * D            # 4096 XPP/XT2 cols per batch

BF = mybir.dt.bfloat16
F32 = mybir.dt.float32
AF = mybir.ActivationFunctionType


def emit_body(ctx, tc, ins, outs, dbg, stage=99):
    nc = tc.nc
    xpp, xt2, wovt, mt2, maskc, bobf = ins
    y = outs["y"]

    # ---------------- persistent tensors ----------------
    pp = ctx.enter_context(tc.tile_pool(name="persist", bufs=1))
    XPP = pp.tile([128, B_LOC * XCOL], BF, tag="XPP")
    XT2 = pp.tile([128, B_LOC * XCOL], BF, tag="XT2")
    ZT = pp.tile([128, B_LOC * XCOL], BF, tag="ZT")
    WOVT = pp.tile([128, 8 * E], BF, tag="WOVT")
    OUT2T = pp.tile([128, B_LOC * 8 * RB], BF, tag="OUT2T")
    MASKT = pp.tile([128, 512], BF, tag="MASKT")
    MT2T = pp.tile([128, 128], BF, tag="MT2T")
    BOBF = pp.tile([1, E], BF, tag="BOBF")
    ONES64 = pp.tile([128, D], BF, tag="ONES64")
    ONESROW = pp.tile([1, 512], BF, tag="ONESROW")

    nc.vector.memset(ONES64[:], 1.0)
    nc.vector.memset(ONESROW[:], 1.0)
    # x loads on the SP HWDGE ring (batch order: Z_0 can start ~3us in);
    # weights go on the ACT ring so they don't delay batch 0. Batch 0 is
    # split fine so the first Z chunks / U matmuls start as early as possible.
    for b in range(B_LOC):
        nc.sync.dma_start(XT2[:, b * XCOL:(b + 1) * XCOL], xt2[b])
        nc.sync.dma_start(XPP[:, b * XCOL:(b + 1) * XCOL], xpp[b])
    nc.scalar.dma_start(MT2T[:], mt2)
    nc.scalar.dma_start(MASKT[:], maskc)
    nc.scalar.dma_start(WOVT[:], wovt)
    nc.scalar.dma_start(BOBF[:], bobf.rearrange("(p n) -> p n", p=1))

    if stage < 2:
        return

    # ---------------- pools ----------------
    # PSUM budget (8 banks): eps-pair 2 (bufs=1 x [128,1024]), ops 2 (zps
    # shares), dps 2, fc 2.
    eps_pool = ctx.enter_context(tc.tile_pool(name="eps", bufs=1, space="PSUM"))
    ops_pool = ctx.enter_context(tc.tile_pool(name="ops", bufs=2, space="PSUM"))
    dps_pool = ctx.enter_context(tc.tile_pool(name="dps", bufs=1, space="PSUM"))
    fc_pool = ctx.enter_context(tc.tile_pool(name="fcps", bufs=3, space="PSUM"))
    aex_pool = ctx.enter_context(tc.tile_pool(name="aex", bufs=6))
    am_pool = ctx.enter_context(tc.tile_pool(name="am", bufs=6))
    rden_pool = ctx.enter_context(tc.tile_pool(name="rden", bufs=3))
    fout_pool = ctx.enter_context(tc.tile_pool(name="fout", bufs=2))
    fpart_pool = ctx.enter_context(tc.tile_pool(name="fpart", bufs=1))

    # PE warmup: dummy matmuls fill the initial input-DMA wait and keep the
    # HAM clock gate at 8/8 before the real work lands
    for w in range(16):
        wps = fc_pool.tile([128, 512], F32, tag="fcps")
        nc.tensor.matmul(wps[:], ONESROW[:, 0:128], ONESROW[:],
                         start=True, stop=True)

    fout_state = {}
    fc_parts = [None] * 8

    def fc_half(b, idx, c2s=range(8), partial=None, add=None):
        # one fc half-tile: idx = rt*2 + he; y-DMA fires after he==1.
        # c2s: which OUT2T chunks to accumulate. partial: SBUF tile to stash
        # a partial sum in. add: SBUF partial to add during the final evict.
        rt, he = idx // 2, idx % 2
        fps = fc_pool.tile([128, 512], F32, tag="fcps")
        c2s = list(c2s)
        first = add is not None   # bias already in the partial being added
        if not first:
            nc.tensor.matmul(fps[:], ONESROW[:, 0:128],
                             BOBF[:, he * 512:(he + 1) * 512],
                             start=True, stop=False)
        for c2 in c2s:
            sec = (b * 8 + c2) * 512
            nc.tensor.matmul(
                fps[:],
                OUT2T[:, sec + rt * 128: sec + (rt + 1) * 128],
                WOVT[:, c2 * E + he * 512: c2 * E + he * 512 + 512],
                start=first, stop=(c2 == c2s[-1]),
            )
            first = False
        if partial is not None:
            nc.scalar.copy(partial[:], fps[:])
            return
        if he == 0:
            fo = fout_pool.tile([128, E], F32, tag="fout")
            fout_state[b] = fo
        fo = fout_state[b]
        if add is not None:
            nc.vector.tensor_add(fo[:, he * 512:(he + 1) * 512], fps[:], add[:])
        else:
            nc.scalar.copy(fo[:, he * 512:(he + 1) * 512], fps[:])
        if he == 1:
            row = b * RB + rt * 128
            nc.sync.dma_start(y[row:row + 128, :], fo[:])

    def z_chunk(b, r):
        zps = ops_pool.tile([128, 512], F32, tag="ops")
        nc.tensor.matmul(zps[:], MT2T[:],
                         XT2[:, b * XCOL + r * 512: b * XCOL + (r + 1) * 512],
                         start=True, stop=True)
        nc.any.tensor_copy(ZT[:, b * XCOL + r * 512: b * XCOL + (r + 1) * 512],
                           zps[:])

    for b in range(B_LOC):
        for r in range(XCOL // 512):
            z_chunk(b, r)
        if stage < 3:
            continue

        # ---------------- attention main loop ----------------
        # Column order within a gm tile is (qpar, qhi, h, j): q = qhi*2+qpar
        # lives at aoff(q) = (q%2)*256 + (q//2)*128 (E psum is parity-banked).
        for c in range(NG // 2):        # gm pairs
            dps = dps_pool.tile([128, 512], F32, tag="dps")
            ops = ops_pool.tile([128, 512], F32, tag="ops")
            # eps pair tile: 2 banks; bank=q-parity, cols gmh*256+(q//2)*128
            eps = eps_pool.tile([128, 1024], F32, tag="eps")
            ams = [None, None]
            for gmh in range(2):
                gm = c * 2 + gmh
                for q in range(NQ):
                    half = (q % 2) * 64
                    blk = b * XCOL + (gm * 2 + q // 2) * 128
                    col = (q % 2) * 512 + gmh * 256 + (q // 2) * 128
                    nc.tensor.matmul(
                        eps[:, col:col + 128],
                        ZT[half:half + 64, blk:blk + 128],
                        XT2[half:half + 64, blk:blk + 128],
                        start=True, stop=True,
                        tile_position=(half, 0),
                    )
                if stage < 4:
                    continue
                aex = aex_pool.tile([128, 512], BF, tag="aex")
                eview = eps[:].rearrange("p (par g cc) -> g p par cc",
                                         par=2, g=2)[gmh]
                nc.scalar.activation(aex[:], eview, AF.Exp, scale=1.0 / 32.0)
                am = am_pool.tile([128, 512], BF, tag="am")
                # gm0 mask on gpsimd (slow, overlaps gm1's exp); gm1 on DVE
                if gmh == 0:
                    nc.gpsimd.tensor_mul(am[:], aex[:], MASKT[:])
                else:
                    nc.vector.tensor_mul(am[:], aex[:], MASKT[:])
                ams[gmh] = am
            # fc filler sits exactly in the exp->mask chain wait on the PE
            # FIFO: batch b-1's fc half between this pair's E and U matmuls
            if stage >= 7 and b > 0:
                fc_half(b - 1, c)
            if stage >= 7 and b == B_LOC - 1 and c >= 4:
                for k in range(2):
                    idx = (c - 4) * 2 + k
                    part = fpart_pool.tile([128, 512], F32, tag="fpart%d" % idx)
                    fc_parts[idx] = part
                    fc_half(b, idx, c2s=range(4), partial=part)
            if stage < 5:
                continue
            # U^T matmuls, gm1 first (its DVE mask finishes first)
            for gmh in (1, 0):
                gm = c * 2 + gmh
                pb = gmh * 64
                am = ams[gmh]
                for q in range(NQ):
                    slab = (b * NG + gm) * NQ + q
                    aoff = (q % 2) * 256 + (q // 2) * 128
                    nc.tensor.matmul(
                        ops[pb:pb + 64, aoff:aoff + 128],
                        XPP[:, slab * D:(slab + 1) * D],
                        am[:, aoff:aoff + 128],
                        start=True, stop=True,
                        tile_position=(0, pb),
                    )
                # den matmul: single 512-col stream per gm
                nc.tensor.matmul(dps[pb:pb + 64, :], ONES64[:], am[:],
                                 start=True, stop=True, tile_position=(0, pb))
            if stage < 6:
                continue
            rden = rden_pool.tile([128, 512], F32, tag="rden")
            nc.vector.reciprocal_approx_fast(rden[:], dps[:])
            sec = (b * 8 + c) * 512
            # out2 row r = h*32 + q*8 + j, src col = qpar*256+qhi*128+h*8+j
            # (split by qpar: codegen handles at most 3 free dims per AP)
            for qpar in range(2):
                out_ap = OUT2T[:, sec:sec + 512].rearrange(
                    "p (h qhi qpar j) -> qpar p qhi h j",
                    h=NH, qhi=2, qpar=2, j=NJ)[qpar]
                nc.vector.tensor_mul(out_ap, ops[:, qpar * 256:qpar * 256 + 256],
                                     rden[:, qpar * 256:qpar * 256 + 256])
        if stage < 7:
            continue

    # epilogue: last batch's fc, second half (c2 4..7) + add stashed partials
    if stage >= 7:
        for idx in range(8):
            fc_half(B_LOC - 1, idx, c2s=range(4, 8), add=fc_parts[idx])

    # ---------------- debug dumps ----------------
    for name, T in (("xt2", XT2), ("zt", ZT), ("out2t", OUT2T)):
        if name in dbg:
            nc.sync.dma_start(dbg[name], T[:])


def build(reps=1, debug=(), stage=99):
    nc = bacc.Bacc("TRN2", target_bir_lowering=False, debug=False,
                   num_devices=N_CORES)
    xpp = nc.dram_tensor("xpp", [B_LOC, 128, XCOL], BF, kind="ExternalInput").ap()
    xt2 = nc.dram_tensor("xt2", [B_LOC, 128, XCOL], BF, kind="ExternalInput").ap()
    wovt = nc.dram_tensor("wovt", [128, 8 * E], BF, kind="ExternalInput").ap()
    mt2 = nc.dram_tensor("mt2", [128, 128], BF, kind="ExternalInput").ap()
    maskc = nc.dram_tensor("maskc", [128, 512], BF, kind="ExternalInput").ap()
    bobf = nc.dram_tensor("bobf", [E], BF, kind="ExternalInput").ap()
    y = nc.dram_tensor("y", [B_LOC * RB, E], F32, kind="ExternalOutput").ap()
    dbg = {}
    for name, shape, dt in [
        ("xt2", [128, B_LOC * XCOL], BF),
        ("zt", [128, B_LOC * XCOL], BF),
        ("out2t", [128, B_LOC * 8 * RB], BF),
    ]:
        if name in debug:
            dbg[name] = nc.dram_tensor(name, shape, dt, kind="ExternalOutput").ap()

    ins = (xpp, xt2, wovt, mt2, maskc, bobf)
    outs = {"y": y}
    with tile.TileContext(nc) as tc:
        with ExitStack() as ctx:
            if reps > 1:
                with tc.For_i(0, reps, 1):
                    emit_body(ctx, tc, ins, outs, dbg, stage=stage)
            else:
                emit_body(ctx, tc, ins, outs, dbg, stage=stage)
    nc.compile()
    return nc


def _bf(a):
    import ml_dtypes
    return np.asarray(a, dtype=np.float32).astype(ml_dtypes.bfloat16)


def prepare_in_maps(x, Wq, Wk, Wv, Wo, bo):
    """Host-side staging: layout x shards + fold weights. Returns in_maps."""
    x = np.asarray(x, np.float32)
    Wq = np.asarray(Wq, np.float32)
    Wk = np.asarray(Wk, np.float32)
    Wv = np.asarray(Wv, np.float32)
    Wo = np.asarray(Wo, np.float32)
    bo = np.asarray(bo, np.float32)

    # weights (shared across cores)
    MT = Wk.T @ Wq                      # Z = X @ MT so that E^T = Z X^T
    mt2 = np.zeros((128, 128), np.float32)
    mt2[:64, :64] = MT
    mt2[64:, 64:] = MT
    mt2 = _bf(mt2)
    maskc = _bf(np.tile(np.kron(np.ones((16, 16), np.float32),
                                np.eye(8, dtype=np.float32)), (1, 4)))
    # wov[gm][e, di] = sum_dv Wo[e, gm*64+dv] * Wv[dv, di]
    wov = np.einsum('gev,vd->ged', Wo.reshape(E, NG, D).transpose(1, 0, 2), Wv)
    # wovt[(gm%2)*64 + di, (gm//2)*1024 + e]
    wovt = _bf(np.ascontiguousarray(
        wov.reshape(8, 2, E, D).transpose(1, 3, 0, 2).reshape(128, 8 * E)))
    bobf = _bf(bo)

    in_maps = []
    for core in range(N_CORES):
        xs = x[core * B_LOC:(core + 1) * B_LOC]
        xr = xs.reshape(B_LOC, NQ, NJ, NG, NH, D)       # b q j gm k d
        xpp = _bf(np.ascontiguousarray(
            xr.transpose(0, 4, 2, 3, 1, 5).reshape(B_LOC, 128, XCOL)))
        xr2 = xs.reshape(B_LOC, 2, 2, NJ, NG, NH, D)    # b qhi qpar j gm k d
        xt2 = _bf(np.ascontiguousarray(
            xr2.transpose(0, 2, 6, 4, 1, 5, 3).reshape(B_LOC, 128, XCOL)))
        in_maps.append({
            "xpp": xpp, "xt2": xt2, "wovt": wovt, "mt2": mt2,
            "maskc": maskc, "bobf": bobf,
        })
    return in_maps


def kernel(x, Wq, Wk, Wv, Wo, bo):
    """Full-input entry point: shards batch over 8 cores, returns full output."""
    from concourse.bass_utils import run_bass_kernel_spmd

    nc = build()
    in_maps = prepare_in_maps(x, Wq, Wk, Wv, Wo, bo)
    res = run_bass_kernel_spmd(nc, in_maps, list(range(N_CORES)))
    out = np.concatenate([res.results[c]["y"] for c in range(N_CORES)], axis=0)
    return out.reshape(B_GLOB, RB, E)
